# revision 21
# baseline (speedup 1.0000x reference)
"""Sparse-attention Trainium2 kernel (8 NeuronCores, SPMD, no collectives).

Sharding: 16 heads / 8 cores = 2 heads per core (all 4 batches).
Weights are column-sliced per head group (Wq/Wk/Wv) and row-sliced (Wo);
each core computes its heads' attention end-to-end plus a partial
out = attn_out @ Wo_slice; the host sums the 8 partials (row-parallel
unshard) and adds bo.

Device algorithm per core:
  phase 1: qT/kT [128, 8192] and v [8192, 128] projections from xT tiles.
  phase 2: per (batch, head) unit, per 128-query tile:
    scores = qT.T @ kT (PE, K=64) + rel-pos bias.
    Rel-pos handled as: global add of qE[:,255] (the j <= i+1 constant),
    plus a diagonal-band correction streamed through a DRAM bounce with a
    per-partition-shifted access pattern, plus a constant for j >= i+256.
    Top-64 threshold per row via 8 rounds of DVE max8/match_replace.
    Masked softmax; gating/Z folded into a diagonal matrix used as the
    "identity" operand of the PE transpose of P (so P^T comes out scaled).
    attn@v accumulated over 16 key tiles into PSUM.
  phase 3: out partial = aoT.T @ Wo via PE, DMA to DRAM.
"""
import sys

sys.path.insert(0, "/opt/trn_rl_repo")

import numpy as np

B, N, D = 4, 2048, 1024
H, DH = 16, 64
R = 256
TOPK = 64
HPC = 2                      # heads per core
INNER = HPC * DH             # 128
BN = B * N                   # 8192
NEG = -3.0e38
NCORES = 8

_CACHE = {}


def build_graph(dbg=False):
    import concourse.bass as bass
    import concourse.mybir as mybir
    from concourse import bacc
    from concourse.bass_types import AP
    from concourse.masks import make_identity
    from concourse.tile import TileContext

    fp = mybir.dt.float32
    bf = mybir.dt.bfloat16
    AX = mybir.AxisListType.X
    AF = mybir.ActivationFunctionType
    OP = mybir.AluOpType

    nc = bacc.Bacc()

    xT = nc.declare_dram_parameter("xT", [D, BN], fp, isOutput=False)
    wq = nc.declare_dram_parameter("wq", [D, INNER], fp, isOutput=False)
    wk = nc.declare_dram_parameter("wk", [D, INNER], fp, isOutput=False)
    wv = nc.declare_dram_parameter("wv", [D, INNER], fp, isOutput=False)
    wo = nc.declare_dram_parameter("wo", [INNER, D], fp, isOutput=False)
    relT = nc.declare_dram_parameter("relT", [DH, R], fp, isOutput=False)
    gat = nc.declare_dram_parameter("gat", [BN], fp, isOutput=False)
    out = nc.declare_dram_parameter("out", [BN, D], fp, isOutput=True)
    if dbg:
        dbg_qT = nc.declare_dram_parameter("dbg_qT", [128, BN], fp, isOutput=True)
        dbg_kT = nc.declare_dram_parameter("dbg_kT", [128, BN], fp, isOutput=True)
        dbg_v = nc.declare_dram_parameter("dbg_v", [128, 64 * INNER], fp, isOutput=True)
        dbg_S = nc.declare_dram_parameter("dbg_S", [128, N], fp, isOutput=True)
        dbg_w64 = nc.declare_dram_parameter("dbg_w64", [128, 64], fp, isOutput=True)
        dbg_P = nc.declare_dram_parameter("dbg_P", [128, N], bf, isOutput=True)
        dbg_aoT = nc.declare_dram_parameter("dbg_aoT", [INNER, BN], fp, isOutput=True)

    aoT_dram = nc.dram_tensor("aoT", [INNER, BN], fp)

    NT = BN // 128            # 64 token tiles
    NI = N // 128             # 16 query tiles per unit

    with TileContext(nc) as tc:
        # ---------- persistent SBUF ----------
        with (
            tc.tile_pool(name="persist", bufs=1) as pp,
            tc.tile_pool(name="consts", bufs=1) as cp,
        ):
            qT = pp.tile([128, BN], fp, tag="qT")
            kT = pp.tile([128, BN], fp, tag="kT")
            vsb = pp.tile([128, NT * INNER], bf, tag="vsb")   # v tile t at cols t*128
            wo_sb = pp.tile([INNER, D], fp, tag="wo")
            relT_sb = pp.tile([DH, R], fp, tag="relT")
            g_sb = pp.tile([128, NT], fp, tag="g")
            i01 = cp.tile([128, 128], fp, tag="i01")
            i01b = cp.tile([128, 128], bf, tag="i01b")
            ones = cp.tile([128, 385], fp, tag="ones")

            nc.sync.dma_start(out=wo_sb[:], in_=wo[:])
            nc.sync.dma_start(out=relT_sb[:], in_=relT[:])
            g_src = AP(tensor=gat[:].tensor, offset=0, ap=[[1, 128], [128, NT]])
            nc.sync.dma_start(out=g_sb[:], in_=g_src)
            make_identity(nc, i01[:])
            make_identity(nc, i01b[:])
            nc.vector.memset(ones[:], 1.0)

            # ---------- phase 1: projections ----------
            with (
                tc.tile_pool(name="p1_sbuf", bufs=3) as xp,
                tc.tile_pool(name="p1_w", bufs=1) as wp,
                tc.tile_pool(name="p1_psum", bufs=4, space="PSUM") as p1ps,
                tc.tile_pool(name="p1_psum_v", bufs=4, space="PSUM") as p1psv,
            ):
                wq_sb = wp.tile([128, 8 * INNER], fp, tag="wq")
                wk_sb = wp.tile([128, 8 * INNER], fp, tag="wk")
                wv_sb = wp.tile([128, 8 * INNER], fp, tag="wv")
                # one DMA per weight: dest [p, kc, m] <- src W[kc*128+p, m]
                for wsb, wsrc in ((wq_sb, wq), (wk_sb, wk), (wv_sb, wv)):
                    src = AP(tensor=wsrc[:].tensor, offset=0,
                             ap=[[INNER, 128], [128 * INNER, 8], [1, INNER]])
                    nc.sync.dma_start(out=wsb[:], in_=src)

                for tcn in range(16):            # 512-token chunks
                    t0 = tcn * 512
                    xt = xp.tile([128, 8 * 512], fp, tag="xt")
                    # one DMA: dest [p, kc, t] <- xT[kc*128+p, t0+t]
                    xsrc = AP(tensor=xT[:].tensor, offset=t0,
                              ap=[[BN, 128], [128 * BN, 8], [1, 512]])
                    nc.sync.dma_start(out=xt[:], in_=xsrc)
                    # qT / kT chunk
                    for dst, wsb in ((qT, wq_sb), (kT, wk_sb)):
                        ps = p1ps.tile([128, 512], fp, tag="proj")
                        for kc in range(8):
                            nc.tensor.matmul(
                                ps[:], lhsT=wsb[:, kc * INNER:(kc + 1) * INNER],
                                rhs=xt[:, kc * 512:(kc + 1) * 512],
                                start=(kc == 0), stop=(kc == 7))
                        nc.scalar.copy(out=dst[:, t0:t0 + 512], in_=ps[:])
                    # v chunk: 4 token tiles
                    for tt in range(4):
                        ps = p1psv.tile([128, INNER], fp, tag="projv")
                        for kc in range(8):
                            nc.tensor.matmul(
                                ps[:],
                                lhsT=xt[:, kc * 512 + tt * 128: kc * 512 + (tt + 1) * 128],
                                rhs=wv_sb[:, kc * INNER:(kc + 1) * INNER],
                                start=(kc == 0), stop=(kc == 7))
                        gt = tcn * 4 + tt
                        nc.vector.tensor_copy(
                            out=vsb[:, gt * INNER:(gt + 1) * INNER], in_=ps[:])

            if dbg:
                nc.sync.dma_start(out=dbg_qT[:], in_=qT[:])
                nc.sync.dma_start(out=dbg_kT[:], in_=kT[:])
                nc.sync.dma_start(out=dbg_v[:], in_=vsb[:])

            # ---------- phase 2: attention ----------
            with (
                tc.tile_pool(name="s_pool", bufs=2) as sp,
                tc.tile_pool(name="sc_pool", bufs=1) as scp,
                tc.tile_pool(name="qh_pool", bufs=1) as qhp,
                tc.tile_pool(name="qe_pool", bufs=2) as qep,
                tc.tile_pool(name="f2_pool", bufs=1) as f2p,
                tc.tile_pool(name="td_pool", bufs=1) as tdp,
                tc.tile_pool(name="small", bufs=4) as smp,
                tc.tile_pool(name="pt_pool", bufs=1) as ptp,
                tc.tile_pool(name="av_pool", bufs=2) as avp,
                tc.tile_pool(name="f2dram", bufs=3, space="DRAM") as f2d,
                tc.tile_pool(name="ps_s", bufs=4, space="PSUM") as psS,
                tc.tile_pool(name="ps_qe", bufs=1, space="PSUM") as psQ,
                tc.tile_pool(name="ps_t", bufs=2, space="PSUM") as psT,
                tc.tile_pool(name="ps_av", bufs=1, space="PSUM") as psA,
            ):
                PT = ptp.tile([128, NI * 512], bf, tag="PT")

                for u in range(8):               # unit = (batch, local head)
                    b, hl = u // 2, u % 2
                    toff = b * N
                    hs0 = hl * 64
                    # stage this unit's q/k at partition base 0
                    q_h = qhp.tile([64, N], fp, tag="qh")
                    k_h = qhp.tile([64, N], fp, tag="kh")
                    nc.sync.dma_start(out=q_h[:], in_=qT[hs0:hs0 + 64, toff:toff + N])
                    nc.sync.dma_start(out=k_h[:], in_=kT[hs0:hs0 + 64, toff:toff + N])
                    for ic in range(4):          # 512-query chunks
                        for ii in range(4):
                            I = ic * 4 + ii
                            i0 = I * 128
                            # --- qE ---
                            pq = psQ.tile([128, R], fp, tag="qe")
                            nc.tensor.matmul(
                                pq[:], lhsT=q_h[:, i0:i0 + 128],
                                rhs=relT_sb[:], start=True, stop=True)
                            qe = qep.tile([128, R], fp, tag="qe")
                            nc.scalar.copy(out=qe[:], in_=pq[:])
                            c255 = qe[:, 255:256]
                            sm = smp.tile([128, 8], fp, tag="sm")
                            nc255 = sm[:, 0:1]
                            d0 = sm[:, 1:2]
                            nm = sm[:, 2:3]
                            zrow = sm[:, 3:4]
                            rz = sm[:, 4:5]
                            rr = sm[:, 5:6]
                            nc.vector.tensor_scalar_mul(nc255, c255, -1.0)
                            nc.vector.tensor_sub(d0, qe[:, 0:1], c255)
                            # --- scores ---
                            S = sp.tile([128, N], fp, tag="S")
                            for jc in range(4):
                                ps = psS.tile([128, 512], fp, tag="sc")
                                nc.tensor.matmul(
                                    ps[:], lhsT=q_h[:, i0:i0 + 128],
                                    rhs=k_h[:, jc * 512:(jc + 1) * 512],
                                    start=True, stop=True)
                                nc.scalar.activation(
                                    out=S[:, jc * 512:(jc + 1) * 512], in_=ps[:],
                                    func=AF.Identity, bias=c255, scale=1.0)
                            # --- rel-pos band correction ---
                            f2w = f2p.tile([128, 768], fp, tag="f2w")
                            nc.vector.memset(f2w[:, 0:129], 0.0)
                            nc.scalar.activation(
                                out=f2w[:, 129:383], in_=qe[:, 254:0:-1],
                                func=AF.Identity, bias=nc255, scale=1.0)
                            nc.scalar.activation(
                                out=f2w[:, 383:768], in_=ones[:, 0:385],
                                func=AF.Copy, bias=0.0, scale=d0)
                            f2t = f2d.tile([128 * 768], fp, tag="f2")
                            f2ap = f2t[:]
                            nc.sync.dma_start(
                                out=AP(tensor=f2ap.tensor, offset=f2ap.offset,
                                       ap=[[768, 128], [1, 768]]),
                                in_=f2w[:])
                            cb = I * 128
                            wband = min(640, N - cb)
                            td = tdp.tile([128, 640], fp, tag="td")
                            nc.sync.dma_start(
                                out=td[:, 0:wband],
                                in_=AP(tensor=f2ap.tensor, offset=f2ap.offset + 127,
                                       ap=[[767, 128], [1, wband]]))
                            nc.gpsimd.tensor_add(
                                S[:, cb:cb + wband], S[:, cb:cb + wband], td[:, 0:wband])
                            if cb + 640 < N:
                                nc.gpsimd.tensor_scalar(
                                    out=S[:, cb + 640:N], in0=S[:, cb + 640:N],
                                    scalar1=d0, scalar2=None, op0=OP.add)
                            if dbg and u == 0 and I == 0:
                                nc.sync.dma_start(out=dbg_S[:], in_=S[:])
                            # --- top-64 threshold ---
                            w64 = smp.tile([128, 64], fp, tag="w64")
                            Sc = scp.tile([128, N], fp, tag="Sc")
                            nc.vector.max(out=w64[:, 0:8], in_=S[:])
                            nc.vector.match_replace(
                                out=Sc[:], in_to_replace=w64[:, 0:8],
                                in_values=S[:], imm_value=NEG)
                            for r_ in range(1, 8):
                                nc.vector.max(out=w64[:, r_ * 8:(r_ + 1) * 8], in_=Sc[:])
                                if r_ < 7:
                                    nc.vector.match_replace(
                                        out=Sc[:], in_to_replace=w64[:, r_ * 8:(r_ + 1) * 8],
                                        in_values=Sc[:], imm_value=NEG)
                            tau = w64[:, 63:64]
                            if dbg and u == 0 and I == 0:
                                nc.sync.dma_start(out=dbg_w64[:], in_=w64[:])
                            # --- masked softmax ---
                            # P <- (S >= tau); Sc reused as exp(S - m); P <- P * Sc
                            nc.vector.tensor_scalar_mul(nm, w64[:, 0:1], -1.0)
                            P = sp.tile([128, N], bf, tag="P")
                            nc.gpsimd.tensor_scalar(
                                out=P[:], in0=S[:], scalar1=tau, scalar2=None,
                                op0=OP.is_ge)
                            nc.scalar.activation(
                                out=Sc[:], in_=S[:], func=AF.Exp, bias=nm, scale=1.0)
                            nc.vector.tensor_mul(P[:], P[:], Sc[:])
                            nc.vector.reduce_sum(zrow, P[:], axis=AX)
                            nc.vector.reciprocal(rz, zrow)
                            gcol = g_sb[:, b * 16 + I: b * 16 + I + 1]
                            nc.vector.tensor_mul(rr, rz, gcol)
                            # scale P rows by g/Z (transpose-mode ignores rhs
                            # values, so it can't fold the scaling)
                            nc.vector.tensor_scalar_mul(P[:], P[:], rr)
                            if dbg and u == 0 and I == 0:
                                nc.sync.dma_start(out=dbg_P[:], in_=P[:])
                            # --- transpose P (groups of 4 share a psum bank) ---
                            pt_ps = None
                            for jb in range(NI):
                                q = jb % 4
                                if q == 0:
                                    pt_ps = psT.tile([128, 512], bf, tag="ptps",
                                                     name=f"ptps{u}_{I}_{jb}")
                                nc.tensor.transpose(
                                    pt_ps[:, q * 128:(q + 1) * 128],
                                    in_=P[:, jb * 128:(jb + 1) * 128],
                                    identity=i01b[:])
                                if q == 3:
                                    jg = jb // 4
                                    dst = AP(tensor=PT[:].tensor,
                                             offset=PT[:].offset + (jg * 4) * 512 + ii * 128,
                                             ap=[[PT[:].ap[0][0], 128], [512, 4], [1, 128]])
                                    nc.scalar.copy(out=dst, in_=pt_ps[:])
                        # --- attn @ v for this 512-query chunk ---
                        pav = psA.tile([64, 512], fp, tag="av")
                        for jb in range(NI):
                            gt = b * 16 + jb
                            nc.tensor.matmul(
                                pav[:], lhsT=vsb[:, gt * INNER + hs0: gt * INNER + hs0 + 64],
                                rhs=PT[:, jb * 512:(jb + 1) * 512],
                                start=(jb == 0), stop=(jb == NI - 1))
                        av = avp.tile([64, 512], fp, tag="avsb")
                        nc.scalar.copy(out=av[:], in_=pav[:])
                        nc.sync.dma_start(
                            out=aoT_dram[hs0:hs0 + 64, toff + ic * 512: toff + (ic + 1) * 512],
                            in_=av[:])

            if dbg:
                nc.sync.dma_start(out=dbg_aoT[:], in_=aoT_dram[:])

            # ---------- phase 3: out partial = aoT.T @ Wo ----------
            with (
                tc.tile_pool(name="p3_in", bufs=3) as p3i,
                tc.tile_pool(name="p3_out", bufs=2) as p3o,
                tc.tile_pool(name="p3_psum", bufs=4, space="PSUM") as p3ps,
            ):
                for tt in range(NT):
                    ao = p3i.tile([128, 128], fp, tag="ao")
                    nc.sync.dma_start(out=ao[:], in_=aoT_dram[:, tt * 128:(tt + 1) * 128])
                    ot = p3o.tile([128, D], fp, tag="ot")
                    for hc in range(2):
                        ps = p3ps.tile([128, 512], fp, tag="o")
                        nc.tensor.matmul(ps[:], lhsT=ao[:],
                                         rhs=wo_sb[:, hc * 512:(hc + 1) * 512],
                                         start=True, stop=True)
                        nc.scalar.copy(out=ot[:, hc * 512:(hc + 1) * 512], in_=ps[:])
                    nc.sync.dma_start(out=out[tt * 128:(tt + 1) * 128, :], in_=ot[:])

    return nc


def _get_nc(dbg=False):
    key = ("nc", dbg)
    if key not in _CACHE:
        nc = build_graph(dbg=dbg)
        if not nc.is_finalized():
            nc.finalize()
        _CACHE[key] = nc
    return _CACHE[key]


def kernel(**inputs):
    from concourse.bass_utils import run_bass_kernel_spmd

    x = np.asarray(inputs["x"], np.float32)
    Wq = np.asarray(inputs["Wq"], np.float32)
    Wkv = np.asarray(inputs["Wkv"], np.float32)
    Wo = np.asarray(inputs["Wo"], np.float32)
    rel_emb = np.asarray(inputs["rel_emb"], np.float32)
    gating = np.asarray(inputs["gating_mask"], np.float32).reshape(BN)
    bo = np.asarray(inputs["bo"], np.float32)
    topk = int(np.asarray(inputs.get("sparse_topk", TOPK)))
    assert topk == TOPK, f"kernel hardcodes topk=64, got {topk}"

    scale = DH ** -0.5
    xT = np.ascontiguousarray(x.reshape(BN, D).T)
    relT = np.ascontiguousarray(rel_emb.T)

    in_maps = []
    for c in range(NCORES):
        sl = slice(2 * c * DH, 2 * c * DH + INNER)
        in_maps.append({
            "xT": xT,
            "wq": np.ascontiguousarray(Wq[:, sl] * scale),
            "wk": np.ascontiguousarray(Wkv[:, :H * DH][:, sl]),
            "wv": np.ascontiguousarray(Wkv[:, H * DH:][:, sl]),
            "wo": np.ascontiguousarray(Wo[sl, :]),
            "relT": relT,
            "gat": gating,
        })

    import os
    nc = _get_nc(dbg=bool(os.environ.get("BASSDBG")))
    res = run_bass_kernel_spmd(nc, in_maps, core_ids=list(range(NCORES)))
    parts = res.results
    import os
    if os.environ.get("BASSDBG"):
        np.savez("/root/problem/dbg_core0.npz", **parts[0])
    acc = np.zeros((BN, D), np.float64)
    for r in parts:
        acc += r["out"].astype(np.float64)
    acc += bo
    return acc.reshape(B, N, D).astype(np.float32)


if __name__ == "__main__":
    nc = build_graph()
    print("graph built OK")


# revision 22
# speedup vs baseline: 126.2586x; 126.2586x over previous
"""Sparse-attention Trainium2 kernel (8 NeuronCores, SPMD, no collectives).

Sharding: 16 heads / 8 cores = 2 heads per core (all 4 batches).
Weights are column-sliced per head group (Wq/Wk/Wv) and row-sliced (Wo);
each core computes its heads' attention end-to-end plus a partial
out = attn_out @ Wo_slice; the host sums the 8 partials (row-parallel
unshard) and adds bo.

Device algorithm per core:
  phase 1: qT/kT [128, 8192] and v [8192, 128] projections from xT tiles.
  phase 2: per (batch, head) unit, per 128-query tile:
    scores = qT.T @ kT (PE, K=64) + rel-pos bias.
    Rel-pos handled as: global add of qE[:,255] (the j <= i+1 constant),
    plus a diagonal-band correction streamed through a DRAM bounce with a
    per-partition-shifted access pattern, plus a constant for j >= i+256.
    Top-64 threshold per row via 8 rounds of DVE max8/match_replace.
    Masked softmax; gating/Z folded into a diagonal matrix used as the
    "identity" operand of the PE transpose of P (so P^T comes out scaled).
    attn@v accumulated over 16 key tiles into PSUM.
  phase 3: out partial = aoT.T @ Wo via PE, DMA to DRAM.
"""
import sys

sys.path.insert(0, "/opt/trn_rl_repo")

import numpy as np

B, N, D = 4, 2048, 1024
H, DH = 16, 64
R = 256
TOPK = 64
HPC = 2                      # heads per core
INNER = HPC * DH             # 128
BN = B * N                   # 8192
NEG = -3.0e38
NCORES = 8

_CACHE = {}


def build_graph(dbg=False):
    import concourse.bass as bass
    import concourse.mybir as mybir
    from concourse import bacc
    from concourse.bass_types import AP
    from concourse.masks import make_identity
    from concourse.tile import TileContext

    fp = mybir.dt.float32
    bf = mybir.dt.bfloat16
    AX = mybir.AxisListType.X
    AF = mybir.ActivationFunctionType
    OP = mybir.AluOpType

    nc = bacc.Bacc()

    xT = nc.declare_dram_parameter("xT", [D, BN], fp, isOutput=False)
    wq = nc.declare_dram_parameter("wq", [D, INNER], fp, isOutput=False)
    wk = nc.declare_dram_parameter("wk", [D, INNER], fp, isOutput=False)
    wv = nc.declare_dram_parameter("wv", [D, INNER], fp, isOutput=False)
    wo = nc.declare_dram_parameter("wo", [INNER, D], fp, isOutput=False)
    relT = nc.declare_dram_parameter("relT", [DH, R], fp, isOutput=False)
    gat = nc.declare_dram_parameter("gat", [BN], fp, isOutput=False)
    out = nc.declare_dram_parameter("out", [BN, D], fp, isOutput=True)
    if dbg:
        dbg_qT = nc.declare_dram_parameter("dbg_qT", [128, BN], fp, isOutput=True)
        dbg_kT = nc.declare_dram_parameter("dbg_kT", [128, BN], fp, isOutput=True)
        dbg_v = nc.declare_dram_parameter("dbg_v", [128, 64 * INNER], fp, isOutput=True)
        dbg_S = nc.declare_dram_parameter("dbg_S", [128, N], fp, isOutput=True)
        dbg_w64 = nc.declare_dram_parameter("dbg_w64", [128, 64], fp, isOutput=True)
        dbg_P = nc.declare_dram_parameter("dbg_P", [128, N], bf, isOutput=True)
        dbg_aoT = nc.declare_dram_parameter("dbg_aoT", [INNER, BN], fp, isOutput=True)

    aoT_dram = nc.dram_tensor("aoT", [INNER, BN], fp)

    NT = BN // 128            # 64 token tiles
    NI = N // 128             # 16 query tiles per unit

    with TileContext(nc) as tc:
        # ---------- persistent SBUF ----------
        with (
            tc.tile_pool(name="persist", bufs=1) as pp,
            tc.tile_pool(name="consts", bufs=1) as cp,
        ):
            qT = pp.tile([128, BN], fp, tag="qT")
            kT = pp.tile([128, BN], fp, tag="kT")
            vsb = pp.tile([128, NT * INNER], bf, tag="vsb")   # v tile t at cols t*128
            wo_sb = pp.tile([INNER, D], fp, tag="wo")
            relT_sb = pp.tile([DH, R], fp, tag="relT")
            g_sb = pp.tile([128, NT], fp, tag="g")
            i01 = cp.tile([128, 128], fp, tag="i01")
            i01b = cp.tile([128, 128], bf, tag="i01b")
            ones = cp.tile([128, 385], fp, tag="ones")

            nc.sync.dma_start(out=wo_sb[:], in_=wo[:])
            nc.sync.dma_start(out=relT_sb[:], in_=relT[:])
            g_src = AP(tensor=gat[:].tensor, offset=0, ap=[[1, 128], [128, NT]])
            nc.sync.dma_start(out=g_sb[:], in_=g_src)
            make_identity(nc, i01[:])
            make_identity(nc, i01b[:])
            nc.vector.memset(ones[:], 1.0)

            # ---------- phase 1: projections ----------
            with (
                tc.tile_pool(name="p1_sbuf", bufs=3) as xp,
                tc.tile_pool(name="p1_w", bufs=1) as wp,
                tc.tile_pool(name="p1_psum", bufs=4, space="PSUM") as p1ps,
                tc.tile_pool(name="p1_psum_v", bufs=4, space="PSUM") as p1psv,
            ):
                wq_sb = wp.tile([128, 8 * INNER], fp, tag="wq")
                wk_sb = wp.tile([128, 8 * INNER], fp, tag="wk")
                wv_sb = wp.tile([128, 8 * INNER], fp, tag="wv")
                # one DMA per weight: dest [p, kc, m] <- src W[kc*128+p, m]
                for wsb, wsrc in ((wq_sb, wq), (wk_sb, wk), (wv_sb, wv)):
                    src = AP(tensor=wsrc[:].tensor, offset=0,
                             ap=[[INNER, 128], [128 * INNER, 8], [1, INNER]])
                    nc.sync.dma_start(out=wsb[:], in_=src)

                for tcn in range(16):            # 512-token chunks
                    t0 = tcn * 512
                    xt = xp.tile([128, 8 * 512], fp, tag="xt")
                    # one DMA: dest [p, kc, t] <- xT[kc*128+p, t0+t]
                    xsrc = AP(tensor=xT[:].tensor, offset=t0,
                              ap=[[BN, 128], [128 * BN, 8], [1, 512]])
                    nc.sync.dma_start(out=xt[:], in_=xsrc)
                    # qT / kT chunk
                    for dst, wsb in ((qT, wq_sb), (kT, wk_sb)):
                        ps = p1ps.tile([128, 512], fp, tag="proj")
                        for kc in range(8):
                            nc.tensor.matmul(
                                ps[:], lhsT=wsb[:, kc * INNER:(kc + 1) * INNER],
                                rhs=xt[:, kc * 512:(kc + 1) * 512],
                                start=(kc == 0), stop=(kc == 7))
                        nc.scalar.copy(out=dst[:, t0:t0 + 512], in_=ps[:])
                    # v chunk: 4 token tiles
                    for tt in range(4):
                        ps = p1psv.tile([128, INNER], fp, tag="projv")
                        for kc in range(8):
                            nc.tensor.matmul(
                                ps[:],
                                lhsT=xt[:, kc * 512 + tt * 128: kc * 512 + (tt + 1) * 128],
                                rhs=wv_sb[:, kc * INNER:(kc + 1) * INNER],
                                start=(kc == 0), stop=(kc == 7))
                        gt = tcn * 4 + tt
                        nc.vector.tensor_copy(
                            out=vsb[:, gt * INNER:(gt + 1) * INNER], in_=ps[:])

            if dbg:
                nc.sync.dma_start(out=dbg_qT[:], in_=qT[:])
                nc.sync.dma_start(out=dbg_kT[:], in_=kT[:])
                nc.sync.dma_start(out=dbg_v[:], in_=vsb[:])

            # ---------- phase 2: attention ----------
            with (
                tc.tile_pool(name="s_pool", bufs=2) as sp,
                tc.tile_pool(name="sc_pool", bufs=1) as scp,
                tc.tile_pool(name="qh_pool", bufs=1) as qhp,
                tc.tile_pool(name="qe_pool", bufs=2) as qep,
                tc.tile_pool(name="f2_pool", bufs=1) as f2p,
                tc.tile_pool(name="td_pool", bufs=1) as tdp,
                tc.tile_pool(name="small", bufs=4) as smp,
                tc.tile_pool(name="pt_pool", bufs=1) as ptp,
                tc.tile_pool(name="av_pool", bufs=2) as avp,
                tc.tile_pool(name="f2dram", bufs=3, space="DRAM") as f2d,
                tc.tile_pool(name="ps_s", bufs=4, space="PSUM") as psS,
                tc.tile_pool(name="ps_qe", bufs=1, space="PSUM") as psQ,
                tc.tile_pool(name="ps_t", bufs=2, space="PSUM") as psT,
                tc.tile_pool(name="ps_av", bufs=1, space="PSUM") as psA,
            ):
                PT = ptp.tile([128, NI * 512], bf, tag="PT")

                for u in range(8):               # unit = (batch, local head)
                    b, hl = u // 2, u % 2
                    toff = b * N
                    hs0 = hl * 64
                    # stage this unit's q/k at partition base 0
                    q_h = qhp.tile([64, N], fp, tag="qh")
                    k_h = qhp.tile([64, N], fp, tag="kh")
                    nc.sync.dma_start(out=q_h[:], in_=qT[hs0:hs0 + 64, toff:toff + N])
                    nc.sync.dma_start(out=k_h[:], in_=kT[hs0:hs0 + 64, toff:toff + N])
                    for ic in range(4):          # 512-query chunks
                        for ii in range(4):
                            I = ic * 4 + ii
                            i0 = I * 128
                            # --- qE ---
                            pq = psQ.tile([128, R], fp, tag="qe")
                            nc.tensor.matmul(
                                pq[:], lhsT=q_h[:, i0:i0 + 128],
                                rhs=relT_sb[:], start=True, stop=True)
                            qe = qep.tile([128, R], fp, tag="qe")
                            nc.scalar.copy(out=qe[:], in_=pq[:])
                            c255 = qe[:, 255:256]
                            sm = smp.tile([128, 8], fp, tag="sm")
                            nc255 = sm[:, 0:1]
                            d0 = sm[:, 1:2]
                            nm = sm[:, 2:3]
                            zrow = sm[:, 3:4]
                            rz = sm[:, 4:5]
                            rr = sm[:, 5:6]
                            nc.vector.tensor_scalar_mul(nc255, c255, -1.0)
                            nc.vector.tensor_sub(d0, qe[:, 0:1], c255)
                            # --- scores ---
                            S = sp.tile([128, N], fp, tag="S")
                            for jc in range(4):
                                ps = psS.tile([128, 512], fp, tag="sc")
                                nc.tensor.matmul(
                                    ps[:], lhsT=q_h[:, i0:i0 + 128],
                                    rhs=k_h[:, jc * 512:(jc + 1) * 512],
                                    start=True, stop=True)
                                nc.scalar.activation(
                                    out=S[:, jc * 512:(jc + 1) * 512], in_=ps[:],
                                    func=AF.Identity, bias=c255, scale=1.0)
                            # --- rel-pos band correction ---
                            f2w = f2p.tile([128, 768], fp, tag="f2w")
                            nc.vector.memset(f2w[:, 0:129], 0.0)
                            nc.scalar.activation(
                                out=f2w[:, 129:383], in_=qe[:, 254:0:-1],
                                func=AF.Identity, bias=nc255, scale=1.0)
                            nc.scalar.activation(
                                out=f2w[:, 383:768], in_=ones[:, 0:385],
                                func=AF.Copy, bias=0.0, scale=d0)
                            f2t = f2d.tile([128 * 768], fp, tag="f2")
                            f2ap = f2t[:]
                            nc.sync.dma_start(
                                out=AP(tensor=f2ap.tensor, offset=f2ap.offset,
                                       ap=[[768, 128], [1, 768]]),
                                in_=f2w[:])
                            cb = I * 128
                            wband = min(640, N - cb)
                            td = tdp.tile([128, 640], fp, tag="td")
                            nc.sync.dma_start(
                                out=td[:, 0:wband],
                                in_=AP(tensor=f2ap.tensor, offset=f2ap.offset + 127,
                                       ap=[[767, 128], [1, wband]]))
                            nc.gpsimd.tensor_add(
                                S[:, cb:cb + wband], S[:, cb:cb + wband], td[:, 0:wband])
                            if cb + 640 < N:
                                nc.gpsimd.tensor_scalar(
                                    out=S[:, cb + 640:N], in0=S[:, cb + 640:N],
                                    scalar1=d0, scalar2=None, op0=OP.add)
                            if dbg and u == 0 and I == 0:
                                nc.sync.dma_start(out=dbg_S[:], in_=S[:])
                            # --- top-64 threshold ---
                            w64 = smp.tile([128, 64], fp, tag="w64")
                            Sc = scp.tile([128, N], fp, tag="Sc")
                            nc.vector.max(out=w64[:, 0:8], in_=S[:])
                            nc.vector.match_replace(
                                out=Sc[:], in_to_replace=w64[:, 0:8],
                                in_values=S[:], imm_value=NEG)
                            for r_ in range(1, 8):
                                nc.vector.max(out=w64[:, r_ * 8:(r_ + 1) * 8], in_=Sc[:])
                                if r_ < 7:
                                    nc.vector.match_replace(
                                        out=Sc[:], in_to_replace=w64[:, r_ * 8:(r_ + 1) * 8],
                                        in_values=Sc[:], imm_value=NEG)
                            tau = w64[:, 63:64]
                            if dbg and u == 0 and I == 0:
                                nc.sync.dma_start(out=dbg_w64[:], in_=w64[:])
                            # --- masked softmax ---
                            # P <- (S >= tau); Sc reused as exp(S - m); P <- P * Sc
                            nc.vector.tensor_scalar_mul(nm, w64[:, 0:1], -1.0)
                            P = sp.tile([128, N], bf, tag="P")
                            nc.gpsimd.tensor_scalar(
                                out=P[:], in0=S[:], scalar1=tau, scalar2=None,
                                op0=OP.is_ge)
                            nc.scalar.activation(
                                out=Sc[:], in_=S[:], func=AF.Exp, bias=nm, scale=1.0)
                            nc.gpsimd.tensor_mul(P[:], P[:], Sc[:])
                            nc.scalar.activation(
                                out=Sc[:], in_=P[:], func=AF.Copy,
                                accum_out=zrow)
                            nc.vector.reciprocal(rz, zrow)
                            gcol = g_sb[:, b * 16 + I: b * 16 + I + 1]
                            nc.vector.tensor_mul(rr, rz, gcol)
                            # scale P rows by g/Z (transpose-mode ignores rhs
                            # values, so it can't fold the scaling)
                            nc.vector.tensor_scalar_mul(P[:], P[:], rr)
                            if dbg and u == 0 and I == 0:
                                nc.sync.dma_start(out=dbg_P[:], in_=P[:])
                            # --- transpose P (groups of 4 share a psum bank) ---
                            pt_ps = None
                            for jb in range(NI):
                                q = jb % 4
                                if q == 0:
                                    pt_ps = psT.tile([128, 512], bf, tag="ptps",
                                                     name=f"ptps{u}_{I}_{jb}")
                                nc.tensor.transpose(
                                    pt_ps[:, q * 128:(q + 1) * 128],
                                    in_=P[:, jb * 128:(jb + 1) * 128],
                                    identity=i01b[:])
                                if q == 3:
                                    jg = jb // 4
                                    dst = AP(tensor=PT[:].tensor,
                                             offset=PT[:].offset + (jg * 4) * 512 + ii * 128,
                                             ap=[[PT[:].ap[0][0], 128], [512, 4], [1, 128]])
                                    nc.scalar.copy(out=dst, in_=pt_ps[:])
                        # --- attn @ v for this 512-query chunk ---
                        pav = psA.tile([64, 512], fp, tag="av")
                        for jb in range(NI):
                            gt = b * 16 + jb
                            nc.tensor.matmul(
                                pav[:], lhsT=vsb[:, gt * INNER + hs0: gt * INNER + hs0 + 64],
                                rhs=PT[:, jb * 512:(jb + 1) * 512],
                                start=(jb == 0), stop=(jb == NI - 1))
                        av = avp.tile([64, 512], fp, tag="avsb")
                        nc.scalar.copy(out=av[:], in_=pav[:])
                        nc.sync.dma_start(
                            out=aoT_dram[hs0:hs0 + 64, toff + ic * 512: toff + (ic + 1) * 512],
                            in_=av[:])

            if dbg:
                nc.sync.dma_start(out=dbg_aoT[:], in_=aoT_dram[:])

            # ---------- phase 3: out partial = aoT.T @ Wo ----------
            with (
                tc.tile_pool(name="p3_in", bufs=3) as p3i,
                tc.tile_pool(name="p3_out", bufs=2) as p3o,
                tc.tile_pool(name="p3_psum", bufs=4, space="PSUM") as p3ps,
            ):
                for tt in range(NT):
                    ao = p3i.tile([128, 128], fp, tag="ao")
                    nc.sync.dma_start(out=ao[:], in_=aoT_dram[:, tt * 128:(tt + 1) * 128])
                    ot = p3o.tile([128, D], fp, tag="ot")
                    for hc in range(2):
                        ps = p3ps.tile([128, 512], fp, tag="o")
                        nc.tensor.matmul(ps[:], lhsT=ao[:],
                                         rhs=wo_sb[:, hc * 512:(hc + 1) * 512],
                                         start=True, stop=True)
                        nc.scalar.copy(out=ot[:, hc * 512:(hc + 1) * 512], in_=ps[:])
                    nc.sync.dma_start(out=out[tt * 128:(tt + 1) * 128, :], in_=ot[:])

    return nc


def _get_nc(dbg=False):
    key = ("nc", dbg)
    if key not in _CACHE:
        nc = build_graph(dbg=dbg)
        if not nc.is_finalized():
            nc.finalize()
        _CACHE[key] = nc
    return _CACHE[key]


def kernel(**inputs):
    from concourse.bass_utils import run_bass_kernel_spmd

    x = np.asarray(inputs["x"], np.float32)
    Wq = np.asarray(inputs["Wq"], np.float32)
    Wkv = np.asarray(inputs["Wkv"], np.float32)
    Wo = np.asarray(inputs["Wo"], np.float32)
    rel_emb = np.asarray(inputs["rel_emb"], np.float32)
    gating = np.asarray(inputs["gating_mask"], np.float32).reshape(BN)
    bo = np.asarray(inputs["bo"], np.float32)
    topk = int(np.asarray(inputs.get("sparse_topk", TOPK)))
    assert topk == TOPK, f"kernel hardcodes topk=64, got {topk}"

    scale = DH ** -0.5
    xT = np.ascontiguousarray(x.reshape(BN, D).T)
    relT = np.ascontiguousarray(rel_emb.T)

    in_maps = []
    for c in range(NCORES):
        sl = slice(2 * c * DH, 2 * c * DH + INNER)
        in_maps.append({
            "xT": xT,
            "wq": np.ascontiguousarray(Wq[:, sl] * scale),
            "wk": np.ascontiguousarray(Wkv[:, :H * DH][:, sl]),
            "wv": np.ascontiguousarray(Wkv[:, H * DH:][:, sl]),
            "wo": np.ascontiguousarray(Wo[sl, :]),
            "relT": relT,
            "gat": gating,
        })

    import os
    nc = _get_nc(dbg=bool(os.environ.get("BASSDBG")))
    res = run_bass_kernel_spmd(nc, in_maps, core_ids=list(range(NCORES)))
    parts = res.results
    import os
    if os.environ.get("BASSDBG"):
        np.savez("/root/problem/dbg_core0.npz", **parts[0])
    acc = np.zeros((BN, D), np.float64)
    for r in parts:
        acc += r["out"].astype(np.float64)
    acc += bo
    return acc.reshape(B, N, D).astype(np.float32)


if __name__ == "__main__":
    nc = build_graph()
    print("graph built OK")
